# revision 15
# baseline (speedup 1.0000x reference)
"""ContrastLoss kernel for 8 Trainium2 NeuronCores (batch-sharded SPMD).

Per core (B_local=4096 rows, 32 tiles of [128,1000]):
  P1  features -> one-hot (is_equal) -> bf16 matmuls accumulate seg[1000,512] in PSUM
      counts via is_equal+accum over a broadcast label row
  P2  AllReduce seg+counts [1000,513]
  P3  momentum-blend centers, normalize, Cn^T via PE transpose, sim matmul,
      simneg = -(1+sim)*0.4975 -> bf16 in DRAM
  P4  per logits tile: exp(x) accum s1; exp(10x) in-place accum s10;
      q = (t10 * 1/s10) * gather(simneg rows); Ln(q + 1+1e-6) accum w
  P5  CE gather logits[i,l_i]; reduce partials; tiny AllReduce; loss scalar
"""
import time
import numpy as np

N_CORES = 8
B = 32768
BL = B // N_CORES          # 4096
T = BL // 128              # 32 tiles
C = 1000
D = 512
KSIM = 0.4975              # sim scale guard: |simneg| < 1 so Ln arg stays > 0

_CACHE = {}


def _build():
    import concourse.bass as bass
    import concourse.mybir as mybir
    import concourse.tile as tile
    from concourse.masks import make_identity

    AF = mybir.ActivationFunctionType
    OP = mybir.AluOpType
    f32 = mybir.dt.float32
    bf16 = mybir.dt.bfloat16
    i32 = mybir.dt.int32

    nc = bass.Bass()
    logits = nc.dram_tensor("logits", [BL, C], f32, kind="ExternalInput")
    features = nc.dram_tensor("features", [BL, D], f32, kind="ExternalInput")
    centers = nc.dram_tensor("centers", [C, D], f32, kind="ExternalInput")
    labrow = nc.dram_tensor("labrow", [1, BL], f32, kind="ExternalInput")
    labf = nc.dram_tensor("labf", [128, T], f32, kind="ExternalInput")
    labi = nc.dram_tensor("labi", [128, T], i32, kind="ExternalInput")
    ceoff = nc.dram_tensor("ceoff", [128, T], i32, kind="ExternalInput")
    iotac = nc.dram_tensor("iotac", [1, C], f32, kind="ExternalInput")
    iotak_in = nc.dram_tensor("iotak", [128, 8], f32, kind="ExternalInput")
    loss_out = nc.dram_tensor("loss", [1, 1], f32, kind="ExternalOutput")
    dbg_s1 = nc.dram_tensor("dbg_s1", [128, T], f32, kind="ExternalOutput")
    dbg_s10 = nc.dram_tensor("dbg_s10", [128, T], f32, kind="ExternalOutput")
    dbg_w = nc.dram_tensor("dbg_w", [128, T], f32, kind="ExternalOutput")
    dbg_cnt = nc.dram_tensor("dbg_cnt", [128, 8], f32, kind="ExternalOutput")
    dbg_rn = nc.dram_tensor("dbg_rn", [128, 8], f32, kind="ExternalOutput")
    dbg_pt = nc.dram_tensor("dbg_pt", [1, 4], f32, kind="ExternalOutput")
    dbg_ceg = nc.dram_tensor("dbg_ceg", [128, T], f32, kind="ExternalOutput")
    dbg_g = nc.dram_tensor("dbg_g", [128, C], f32, kind="ExternalOutput")
    dbg_sn = nc.dram_tensor("dbg_sn", [128, C], f32, kind="ExternalOutput")
    dbg_u = nc.dram_tensor("dbg_u", [128, D], f32, kind="ExternalOutput")
    dbg_ct = nc.dram_tensor("dbg_ct", [128, C], f32, kind="ExternalOutput")
    dbg_sp = nc.dram_tensor("dbg_sp", [128, C], f32, kind="ExternalOutput")

    groups = [list(range(N_CORES))]
    CS = [128] * 7 + [104]          # class chunks, 128-aligned offsets
    CO = [128 * i for i in range(8)]

    with tile.TileContext(nc) as tc:
        with (
            tc.tile_pool(name="dram", bufs=1, space="DRAM") as dram,
            tc.tile_pool(name="singles", bufs=1) as sg,
            tc.tile_pool(name="lp", bufs=8) as lp,
            tc.tile_pool(name="fp", bufs=3) as fp,
            tc.tile_pool(name="fb", bufs=3) as fbp,
            tc.tile_pool(name="oh", bufs=3) as ohp,
            tc.tile_pool(name="gp", bufs=3) as gpp,
            tc.tile_pool(name="disc", bufs=2) as dcp,
            tc.tile_pool(name="cw", bufs=2) as cwp,
        ):
            arbuf = dram.tile([C, D + 1], f32)
            arbuf2 = dram.tile([C, D + 1], f32)
            simneg = dram.tile([C, C], bf16)
            pin = dram.tile([1, 4], f32)
            pout = dram.tile([1, 4], f32)

            # ---- constants / small loads ----
            iob = sg.tile([128, C], f32)
            nc.sync.dma_start(out=iob[:], in_=bass.AP(iotac, 0, [[0, 128], [1, C]]))
            labb = sg.tile([128, BL], f32)
            nc.sync.dma_start(out=labb[:], in_=bass.AP(labrow, 0, [[0, 128], [1, BL]]))
            labft = sg.tile([128, T], f32)
            nc.sync.dma_start(out=labft[:], in_=labf[:])
            labit = sg.tile([128, T], i32)
            nc.sync.dma_start(out=labit[:], in_=labi[:])
            ceofft = sg.tile([128, T], i32)
            nc.sync.dma_start(out=ceofft[:], in_=ceoff[:])
            eps1 = sg.tile([128, 1], f32)
            nc.vector.memset(eps1[:], 1.0 + 1e-6)
            ident = sg.tile([128, 128], bf16)
            make_identity(nc, ident[:])
            s1col = sg.tile([128, T], f32)
            s10col = sg.tile([128, T], f32)
            wcol = sg.tile([128, T], f32)
            nrm2 = sg.tile([128, 8], f32)
            nc.vector.memset(nrm2[:], 1.0)
            counts = sg.tile([128, 8], f32)
            nc.vector.memset(counts[:], 0.0)

            # ---- logits DMA (ACT hwdge queue), resident ----
            xts = []
            for t in range(T):
                xt = lp.tile([128, C], f32)
                nc.scalar.dma_start(out=xt[:], in_=logits[128 * t:128 * (t + 1), :])
                xts.append(xt)

            # ---- P1: segment-sum matmuls ----
            segps_cm = tc.tile_pool(name="seg_ps", bufs=1, space="PSUM")
            segps = segps_cm.__enter__()
            seg_acc = [segps.tile([128, D], f32, space="PSUM", name=f"seg{i}",
                      tag=f"seg{i}") for i in range(8)]
            for t in range(T):
                ft = fp.tile([128, D], f32)
                nc.sync.dma_start(out=ft[:], in_=features[128 * t:128 * (t + 1), :])
                fb = fbp.tile([128, D], bf16)
                nc.vector.tensor_copy(out=fb[:], in_=ft[:])
                oh = ohp.tile([128, C], bf16)
                nc.vector.tensor_scalar(
                    out=oh[:], in0=iob[:], scalar1=labft[:, t:t + 1], scalar2=None,
                    op0=OP.is_equal)
                for cc in range(8):
                    nc.tensor.matmul(
                        out=seg_acc[cc][:CS[cc], :],
                        lhsT=oh[:, CO[cc]:CO[cc] + CS[cc]],
                        rhs=fb[:], start=(t == 0), stop=(t == T - 1))

            # ---- P1b: counts (8 chunks of 128 classes) ----
            cscr = sg.tile([128, BL], bf16)
            iotak = sg.tile([128, 8], f32)
            nc.sync.dma_start(out=iotak[:], in_=iotak_in[:])
            for c in range(8):
                nc.vector.tensor_scalar(
                    out=cscr[:], in0=labb[:], scalar1=iotak[:, c:c + 1], scalar2=None,
                    op0=OP.is_equal)
                nc.vector.tensor_reduce(out=counts[:, c:c + 1], in_=cscr[:],
                                        axis=mybir.AxisListType.X, op=OP.add)

            # ---- P2: seg+counts -> DRAM, AllReduce ----
            for cc in range(8):
                ssb = cwp.tile([128, D], f32)
                nc.vector.tensor_copy(out=ssb[:CS[cc], :], in_=seg_acc[cc][:CS[cc], :])
                nc.sync.dma_start(out=arbuf[CO[cc]:CO[cc] + CS[cc], 0:D],
                                  in_=ssb[:CS[cc], :])
            for c in range(8):
                rows = min(128, C - 128 * c)
                nc.sync.dma_start(
                    out=arbuf[128 * c:128 * c + rows, D:D + 1],
                    in_=counts[:rows, c:c + 1])
            segps_cm.__exit__(None, None, None)
            nc.gpsimd.collective_compute(
                "AllReduce", OP.add, replica_groups=groups,
                ins=[arbuf.opt()], outs=[arbuf2.opt()])

            # ---- P3: centers update + normalize ----
            Us = []
            for cc in range(8):
                n = CS[cc]
                ar = cwp.tile([128, D + 1], f32)
                nc.sync.dma_start(out=ar[:n, :], in_=arbuf2[CO[cc]:CO[cc] + n, :])
                cent = cwp.tile([128, D], f32)
                nc.sync.dma_start(out=cent[:n, :], in_=centers[CO[cc]:CO[cc] + n, :])
                cw = ar[:n, D:D + 1]
                sc = cwp.tile([128, 1], f32)
                nc.vector.tensor_scalar_max(sc[:n, :], cw, 1.0)
                r = cwp.tile([128, 1], f32)
                nc.vector.reciprocal(out=r[:n, :], in_=sc[:n, :])
                pm = cwp.tile([128, 1], f32)
                nc.vector.tensor_scalar(
                    out=pm[:n, :], in0=cw, scalar1=0.0, scalar2=0.1,
                    op0=OP.is_gt, op1=OP.mult)
                u = cwp.tile([128, D], f32)
                nc.vector.tensor_scalar_mul(u[:n, :], ar[:n, 0:D], r[:n, 0:1])
                d = cwp.tile([128, D], f32)
                nc.vector.tensor_tensor(out=d[:n, :], in0=u[:n, :], in1=cent[:n, :],
                                        op=OP.subtract)
                U = cwp.tile([128, D], f32, tag=f"U{cc}", bufs=1)
                nc.vector.scalar_tensor_tensor(
                    out=U[:n, :], in0=d[:n, :], scalar=pm[:n, 0:1], in1=cent[:n, :],
                    op0=OP.mult, op1=OP.add)
                scr = cwp.tile([128, D], f32, tag="nscr")
                nc.scalar.activation(out=scr[:n, :], in_=U[:n, :], func=AF.Square,
                                     accum_out=nrm2[:n, cc:cc + 1])
                Us.append(U)
            nc.sync.dma_start(out=dbg_u[:], in_=Us[0][:])
            nrm = sg.tile([128, 8], f32)
            nc.scalar.activation(out=nrm[:], in_=nrm2[:], func=AF.Sqrt)
            rn = sg.tile([128, 8], f32)
            nc.vector.reciprocal(out=rn[:], in_=nrm[:])
            Cns = []
            for cc in range(8):
                n = CS[cc]
                Cn = cwp.tile([128, D], bf16, tag=f"Cn{cc}", bufs=1)
                nc.vector.tensor_scalar_mul(Cn[:n, :], Us[cc][:n, :], rn[:n, cc:cc + 1])
                Cns.append(Cn)

            # ---- P3c: transpose Cn -> CnT [512,1000] bf16 (4 tiles [128,1000]) ----
            ctps_cm = tc.tile_pool(name="ct_ps", bufs=2, space="PSUM")
            ctps = ctps_cm.__enter__()
            simps_cm = tc.tile_pool(name="sim_ps", bufs=3, space="PSUM")
            simps = simps_cm.__enter__()
            CnTs = []
            for fc in range(4):
                ctp = ctps.tile([128, C], bf16, space="PSUM")
                for cc in range(8):
                    n = CS[cc]
                    nc.tensor.transpose(
                        out=ctp[:, CO[cc]:CO[cc] + n],
                        in_=Cns[cc][:n, 128 * fc:128 * (fc + 1)],
                        identity=ident[:n, :n])
                ct = sg.tile([128, C], bf16, tag=f"CnT{fc}", bufs=1)
                nc.vector.tensor_copy(out=ct[:], in_=ctp[:])
                CnTs.append(ct)

            ctf = cwp.tile([128, C], f32, tag="ctf", bufs=1)
            nc.vector.tensor_copy(out=ctf[:], in_=CnTs[0][:])
            nc.sync.dma_start(out=dbg_ct[:], in_=ctf[:])
            # ---- P3d: sim matmul + simneg -> DRAM ----
            for mc in range(8):
                m = CS[mc]
                sn = cwp.tile([128, C], bf16, tag="snsb")
                for nh in range(2):
                    sp = simps.tile([128, 500], f32, space="PSUM", name=f"sp{mc}_{nh}",
                                    tag="sp")
                    for kc in range(4):
                        nc.tensor.matmul(
                            out=sp[:m, :],
                            lhsT=CnTs[kc][:, CO[mc]:CO[mc] + m],
                            rhs=CnTs[kc][:, 500 * nh:500 * (nh + 1)],
                            start=(kc == 0), stop=(kc == 3))
                    if mc == 0:
                        spf = cwp.tile([128, 500], f32, tag="spf", bufs=1)
                        nc.vector.tensor_copy(out=spf[:m, :], in_=sp[:m, :])
                        nc.sync.dma_start(out=dbg_sp[:, 500 * nh:500 * (nh + 1)],
                                          in_=spf[:])
                    nc.vector.tensor_scalar(
                        out=sn[:m, 500 * nh:500 * (nh + 1)], in0=sp[:m, :],
                        scalar1=-KSIM, scalar2=-KSIM,
                        op0=OP.mult, op1=OP.add)
                nc.sync.dma_start(out=simneg[CO[mc]:CO[mc] + m, :], in_=sn[:m, :])

            simps_cm.__exit__(None, None, None)
            ctps_cm.__exit__(None, None, None)
            # ---- P4: logits passes ----
            for t in range(T):
                xt = xts[t]
                dc = dcp.tile([128, C], bf16)
                nc.scalar.activation(out=dc[:], in_=xt[:], func=AF.Exp,
                                     accum_out=s1col[:, t:t + 1])
                nc.scalar.activation(out=xt[:], in_=xt[:], func=AF.Exp, scale=10.0,
                                     accum_out=s10col[:, t:t + 1])
                rc = cwp.tile([128, 1], f32, tag="rc")
                nc.vector.reciprocal(out=rc[:], in_=s10col[:, t:t + 1])
                g = gpp.tile([128, C], bf16)
                nc.gpsimd.indirect_dma_start(
                    out=g[:], out_offset=None, in_=simneg[:],
                    in_offset=bass.IndirectOffsetOnAxis(ap=labit[:, t:t + 1], axis=0))
                if t == 0:
                    gf = cwp.tile([128, C], f32, tag="gf", bufs=1)
                    nc.vector.tensor_copy(out=gf[:], in_=g[:])
                    nc.sync.dma_start(out=dbg_g[:], in_=gf[:])
                nc.vector.scalar_tensor_tensor(
                    out=xt[:], in0=xt[:], scalar=rc[:, 0:1], in1=g[:],
                    op0=OP.mult, op1=OP.mult)
                dc2 = dcp.tile([128, C], bf16)
                nc.scalar.activation(out=dc2[:], in_=xt[:], func=AF.Ln,
                                     bias=eps1[:, 0:1],
                                     accum_out=wcol[:, t:t + 1])

            # ---- P5: CE gather + final reduction ----
            ceg = sg.tile([128, T], f32)
            logit_flat = bass.AP(logits, 0, [[1, BL * C], [1, 1]])
            for t in range(T):
                nc.gpsimd.indirect_dma_start(
                    out=ceg[:, t:t + 1], out_offset=None, in_=logit_flat,
                    in_offset=bass.IndirectOffsetOnAxis(ap=ceofft[:, t:t + 1], axis=0))
            lnscr = sg.tile([128, T], f32)
            a = sg.tile([128, 4], f32)
            nc.vector.memset(a[:], 0.0)
            nc.scalar.activation(out=lnscr[:], in_=s1col[:], func=AF.Ln,
                                 accum_out=a[:, 0:1])
            nc.vector.tensor_reduce(out=a[:, 1:2], in_=ceg[:],
                                    axis=mybir.AxisListType.X, op=OP.add)
            nc.vector.tensor_reduce(out=a[:, 2:3], in_=wcol[:],
                                    axis=mybir.AxisListType.X, op=OP.add)
            pr = sg.tile([1, 4], f32)
            nc.gpsimd.tensor_reduce(out=pr[:1, :], in_=a[:],
                                    axis=mybir.AxisListType.C, op=OP.add)
            nc.sync.dma_start(out=pin[:], in_=pr[:1, :])
            nc.gpsimd.collective_compute(
                "AllReduce", OP.add, replica_groups=groups,
                ins=[pin.opt()], outs=[pout.opt()])
            pt = sg.tile([1, 4], f32)
            nc.sync.dma_start(out=pt[:1, :], in_=pout[:])
            # loss = (sum_lns1 - sum_xg)/B - 0.1*sum_w/(B*C)
            dl = sg.tile([1, 1], f32)
            nc.vector.tensor_tensor(out=dl[:1, :], in0=pt[:1, 0:1], in1=pt[:1, 1:2],
                                    op=OP.subtract)
            nc.vector.tensor_scalar_mul(dl[:1, :], dl[:1, :], 1.0 / B)
            el = sg.tile([1, 1], f32)
            nc.vector.tensor_scalar_mul(el[:1, :], pt[:1, 2:3], -0.1 / (B * C))
            fl = sg.tile([1, 1], f32)
            nc.vector.tensor_tensor(out=fl[:1, :], in0=dl[:1, :], in1=el[:1, :],
                                    op=OP.add)
            nc.sync.dma_start(out=loss_out[:], in_=fl[:1, :])
            nc.sync.dma_start(out=dbg_s1[:], in_=s1col[:])
            nc.sync.dma_start(out=dbg_s10[:], in_=s10col[:])
            nc.sync.dma_start(out=dbg_w[:], in_=wcol[:])
            nc.sync.dma_start(out=dbg_cnt[:], in_=counts[:])
            nc.sync.dma_start(out=dbg_rn[:], in_=rn[:])
            nc.sync.dma_start(out=dbg_pt[:], in_=pt[:1, :])
            nc.sync.dma_start(out=dbg_ceg[:], in_=ceg[:])
            snb = cwp.tile([128, C], bf16, tag="snb2", bufs=1)
            nc.sync.dma_start(out=snb[:], in_=simneg[0:128, :])
            snf = cwp.tile([128, C], f32, tag="snf2", bufs=1)
            nc.vector.tensor_copy(out=snf[:], in_=snb[:])
            nc.sync.dma_start(out=dbg_sn[:], in_=snf[:])
    return nc


def _install_patches():
    """Walrus in this container accepts only one sync-wait per instruction:
    split multi-wait instructions into single-wait NOPs."""
    import sys
    import types
    import concourse.tile as tile
    import concourse.mybir as mybir

    if "bass_patches_inline" in sys.modules:
        return

    def split_multi_waits(nc):
        for f in nc.m.functions:
            for bb in f.blocks:
                insts = list(bb.instructions)
                out = []
                changed = False
                for ins in insts:
                    si = getattr(ins, "sync_info", None)
                    waits = list(si.on_wait) if (si is not None and si.on_wait) else []
                    if len(waits) > 1:
                        for w in waits[:-1]:
                            nop = mybir.InstNoOp(
                                name=nc.get_next_instruction_name(),
                                engine=ins.engine)
                            nop.sync_info = mybir.SyncInfo(on_wait=[w], on_update=[])
                            nc.register_instruction(nop)
                            out.append(nop)
                        ins.sync_info = mybir.SyncInfo(
                            on_wait=[waits[-1]], on_update=list(si.on_update or []))
                        changed = True
                    out.append(ins)
                if changed:
                    try:
                        bb.instructions = out
                    except Exception:
                        while len(bb.instructions):
                            bb.instructions.pop()
                        for x in out:
                            bb.instructions.append(x)

    orig_exit = tile.TileContext.__exit__

    def patched_exit(self, exc_type, exc_value, traceback):
        r = orig_exit(self, exc_type, exc_value, traceback)
        if not exc_type:
            split_multi_waits(self.nc)
        return r

    tile.TileContext.__exit__ = patched_exit
    sys.modules["bass_patches_inline"] = types.ModuleType("bass_patches_inline")


def _prep_inputs(logits, features, labels, class_centers):
    logits = np.ascontiguousarray(np.asarray(logits, dtype=np.float32))
    features = np.ascontiguousarray(np.asarray(features, dtype=np.float32))
    labels = np.asarray(labels).astype(np.int64)
    centers = np.ascontiguousarray(np.asarray(class_centers, dtype=np.float32))
    in_maps = []
    for i in range(N_CORES):
        sl = slice(BL * i, BL * (i + 1))
        lab = labels[sl].astype(np.int32)
        labf = lab.reshape(T, 128).T.astype(np.float32).copy()
        labi = lab.reshape(T, 128).T.astype(np.int32).copy()
        ceoff = (np.arange(BL, dtype=np.int64) * C + lab).astype(np.int32)
        ceoff = ceoff.reshape(T, 128).T.copy()
        in_maps.append({
            "logits": np.ascontiguousarray(logits[sl]),
            "features": np.ascontiguousarray(features[sl]),
            "centers": centers,
            "labrow": lab.astype(np.float32).reshape(1, BL),
            "labf": labf,
            "labi": labi,
            "ceoff": ceoff,
            "iotac": np.arange(C, dtype=np.float32).reshape(1, C),
            "iotak": (np.arange(128, dtype=np.float32)[:, None]
                      + 128.0 * np.arange(8, dtype=np.float32)[None, :]),
        })
    return in_maps


def kernel(**inputs):
    _install_patches()
    from concourse.bass_utils import run_bass_kernel_spmd

    if "nc" not in _CACHE:
        _CACHE["nc"] = _build()
    nc = _CACHE["nc"]
    in_maps = _prep_inputs(
        inputs["logits"], inputs["features"], inputs["labels"],
        inputs["class_centers"])
    t0 = time.perf_counter()
    res = run_bass_kernel_spmd(nc, in_maps, list(range(N_CORES)))
    _CACHE["last_wall_ns"] = (time.perf_counter() - t0) * 1e9
    loss = np.asarray(res.results[0]["loss"], dtype=np.float32).reshape(())
    return loss


# revision 16
# speedup vs baseline: 1.8181x; 1.8181x over previous
"""ContrastLoss kernel for 8 Trainium2 NeuronCores (batch-sharded SPMD).

Per core (B_local=4096 rows, 32 tiles of [128,1000]):
  P1  features -> one-hot (is_equal) -> bf16 matmuls accumulate seg[1000,512] in PSUM
      counts via is_equal+accum over a broadcast label row
  P2  AllReduce seg+counts [1000,513]
  P3  momentum-blend centers, normalize, Cn^T via PE transpose, sim matmul,
      simneg = -(1+sim)*0.4975 -> bf16 in DRAM
  P4  per logits tile: exp(x) accum s1; exp(10x) in-place accum s10;
      q = (t10 * 1/s10) * gather(simneg rows); Ln(q + 1+1e-6) accum w
  P5  CE gather logits[i,l_i]; reduce partials; tiny AllReduce; loss scalar
"""
import time
import numpy as np

N_CORES = 8
B = 32768
BL = B // N_CORES          # 4096
T = BL // 128              # 32 tiles
C = 1000
D = 512
KSIM = 0.4975              # sim scale guard: |simneg| < 1 so Ln arg stays > 0

_CACHE = {}


def _build():
    import concourse.bass as bass
    import concourse.mybir as mybir
    import concourse.tile as tile
    from concourse.masks import make_identity

    AF = mybir.ActivationFunctionType
    OP = mybir.AluOpType
    f32 = mybir.dt.float32
    bf16 = mybir.dt.bfloat16
    i32 = mybir.dt.int32

    nc = bass.Bass()
    logits = nc.dram_tensor("logits", [BL, C], f32, kind="ExternalInput")
    features = nc.dram_tensor("features", [BL, D], f32, kind="ExternalInput")
    centers = nc.dram_tensor("centers", [C, D], f32, kind="ExternalInput")
    labrow = nc.dram_tensor("labrow", [1, BL], f32, kind="ExternalInput")
    labf = nc.dram_tensor("labf", [128, T], f32, kind="ExternalInput")
    labi = nc.dram_tensor("labi", [128, T], i32, kind="ExternalInput")
    ceoff = nc.dram_tensor("ceoff", [128, T], i32, kind="ExternalInput")
    iotac = nc.dram_tensor("iotac", [1, C], f32, kind="ExternalInput")
    iotak_in = nc.dram_tensor("iotak", [128, 8], f32, kind="ExternalInput")
    loss_out = nc.dram_tensor("loss", [1, 1], f32, kind="ExternalOutput")

    groups = [list(range(N_CORES))]
    CS = [128] * 7 + [104]          # class chunks, 128-aligned offsets
    CO = [128 * i for i in range(8)]

    with tile.TileContext(nc) as tc:
        with (
            tc.tile_pool(name="dram", bufs=1, space="DRAM") as dram,
            tc.tile_pool(name="singles", bufs=1) as sg,
            tc.tile_pool(name="lp", bufs=8) as lp,
            tc.tile_pool(name="fp", bufs=3) as fp,
            tc.tile_pool(name="fb", bufs=3) as fbp,
            tc.tile_pool(name="oh", bufs=3) as ohp,
            tc.tile_pool(name="gp", bufs=3) as gpp,
            tc.tile_pool(name="disc", bufs=2) as dcp,
            tc.tile_pool(name="cw", bufs=2) as cwp,
        ):
            arbuf = dram.tile([C, D + 1], f32)
            arbuf2 = dram.tile([C, D + 1], f32)
            simneg = dram.tile([C, C], bf16)
            pin = dram.tile([1, 4], f32)
            pout = dram.tile([1, 4], f32)

            # ---- constants / small loads ----
            iob = sg.tile([128, C], f32)
            nc.sync.dma_start(out=iob[:], in_=bass.AP(iotac, 0, [[0, 128], [1, C]]))
            labb = sg.tile([128, BL], f32)
            nc.sync.dma_start(out=labb[:], in_=bass.AP(labrow, 0, [[0, 128], [1, BL]]))
            labft = sg.tile([128, T], f32)
            nc.sync.dma_start(out=labft[:], in_=labf[:])
            labit = sg.tile([128, T], i32)
            nc.sync.dma_start(out=labit[:], in_=labi[:])
            ceofft = sg.tile([128, T], i32)
            nc.sync.dma_start(out=ceofft[:], in_=ceoff[:])
            eps1 = sg.tile([128, 1], f32)
            nc.vector.memset(eps1[:], 1.0 + 1e-6)
            ident = sg.tile([128, 128], bf16)
            make_identity(nc, ident[:])
            s1col = sg.tile([128, T], f32)
            s10col = sg.tile([128, T], f32)
            wcol = sg.tile([128, T], f32)
            nrm2 = sg.tile([128, 8], f32)
            nc.vector.memset(nrm2[:], 1.0)
            counts = sg.tile([128, 8], f32)
            nc.vector.memset(counts[:], 0.0)

            # ---- logits DMA (ACT hwdge queue), resident ----
            xts = []
            for t in range(T):
                xt = lp.tile([128, C], f32)
                nc.scalar.dma_start(out=xt[:], in_=logits[128 * t:128 * (t + 1), :])
                xts.append(xt)

            # ---- P1: segment-sum matmuls ----
            segps_cm = tc.tile_pool(name="seg_ps", bufs=1, space="PSUM")
            segps = segps_cm.__enter__()
            seg_acc = [segps.tile([128, D], f32, space="PSUM", name=f"seg{i}",
                      tag=f"seg{i}") for i in range(8)]
            for t in range(T):
                ft = fp.tile([128, D], f32)
                nc.sync.dma_start(out=ft[:], in_=features[128 * t:128 * (t + 1), :])
                fb = fbp.tile([128, D], bf16)
                nc.vector.tensor_copy(out=fb[:], in_=ft[:])
                oh = ohp.tile([128, C], bf16)
                nc.vector.tensor_scalar(
                    out=oh[:], in0=iob[:], scalar1=labft[:, t:t + 1], scalar2=None,
                    op0=OP.is_equal)
                for cc in range(8):
                    nc.tensor.matmul(
                        out=seg_acc[cc][:CS[cc], :],
                        lhsT=oh[:, CO[cc]:CO[cc] + CS[cc]],
                        rhs=fb[:], start=(t == 0), stop=(t == T - 1))

            # ---- P1b: counts (8 chunks of 128 classes) ----
            cscr = sg.tile([128, BL], bf16)
            iotak = sg.tile([128, 8], f32)
            nc.sync.dma_start(out=iotak[:], in_=iotak_in[:])
            for c in range(8):
                nc.vector.tensor_scalar(
                    out=cscr[:], in0=labb[:], scalar1=iotak[:, c:c + 1], scalar2=None,
                    op0=OP.is_equal)
                nc.vector.tensor_reduce(out=counts[:, c:c + 1], in_=cscr[:],
                                        axis=mybir.AxisListType.X, op=OP.add)

            # ---- P2: seg+counts -> DRAM, AllReduce ----
            for cc in range(8):
                ssb = cwp.tile([128, D], f32)
                nc.vector.tensor_copy(out=ssb[:CS[cc], :], in_=seg_acc[cc][:CS[cc], :])
                nc.sync.dma_start(out=arbuf[CO[cc]:CO[cc] + CS[cc], 0:D],
                                  in_=ssb[:CS[cc], :])
            for c in range(8):
                rows = min(128, C - 128 * c)
                nc.sync.dma_start(
                    out=arbuf[128 * c:128 * c + rows, D:D + 1],
                    in_=counts[:rows, c:c + 1])
            segps_cm.__exit__(None, None, None)
            nc.gpsimd.collective_compute(
                "AllReduce", OP.add, replica_groups=groups,
                ins=[arbuf.opt()], outs=[arbuf2.opt()])

            # ---- P3: centers update + normalize ----
            Us = []
            for cc in range(8):
                n = CS[cc]
                ar = cwp.tile([128, D + 1], f32)
                nc.sync.dma_start(out=ar[:n, :], in_=arbuf2[CO[cc]:CO[cc] + n, :])
                cent = cwp.tile([128, D], f32)
                nc.sync.dma_start(out=cent[:n, :], in_=centers[CO[cc]:CO[cc] + n, :])
                cw = ar[:n, D:D + 1]
                sc = cwp.tile([128, 1], f32)
                nc.vector.tensor_scalar_max(sc[:n, :], cw, 1.0)
                r = cwp.tile([128, 1], f32)
                nc.vector.reciprocal(out=r[:n, :], in_=sc[:n, :])
                pm = cwp.tile([128, 1], f32)
                nc.vector.tensor_scalar(
                    out=pm[:n, :], in0=cw, scalar1=0.0, scalar2=0.1,
                    op0=OP.is_gt, op1=OP.mult)
                u = cwp.tile([128, D], f32)
                nc.vector.tensor_scalar_mul(u[:n, :], ar[:n, 0:D], r[:n, 0:1])
                d = cwp.tile([128, D], f32)
                nc.vector.tensor_tensor(out=d[:n, :], in0=u[:n, :], in1=cent[:n, :],
                                        op=OP.subtract)
                U = cwp.tile([128, D], f32, tag=f"U{cc}", bufs=1)
                nc.vector.scalar_tensor_tensor(
                    out=U[:n, :], in0=d[:n, :], scalar=pm[:n, 0:1], in1=cent[:n, :],
                    op0=OP.mult, op1=OP.add)
                scr = cwp.tile([128, D], f32, tag="nscr")
                nc.scalar.activation(out=scr[:n, :], in_=U[:n, :], func=AF.Square,
                                     accum_out=nrm2[:n, cc:cc + 1])
                Us.append(U)
            nrm = sg.tile([128, 8], f32)
            nc.scalar.activation(out=nrm[:], in_=nrm2[:], func=AF.Sqrt)
            rn = sg.tile([128, 8], f32)
            nc.vector.reciprocal(out=rn[:], in_=nrm[:])
            Cns = []
            for cc in range(8):
                n = CS[cc]
                Cn = cwp.tile([128, D], bf16, tag=f"Cn{cc}", bufs=1)
                nc.vector.tensor_scalar_mul(Cn[:n, :], Us[cc][:n, :], rn[:n, cc:cc + 1])
                Cns.append(Cn)

            # ---- P3c: transpose Cn -> CnT [512,1000] bf16 (4 tiles [128,1000]) ----
            ctps_cm = tc.tile_pool(name="ct_ps", bufs=2, space="PSUM")
            ctps = ctps_cm.__enter__()
            simps_cm = tc.tile_pool(name="sim_ps", bufs=3, space="PSUM")
            simps = simps_cm.__enter__()
            CnTs = []
            for fc in range(4):
                ctp = ctps.tile([128, C], bf16, space="PSUM")
                for cc in range(8):
                    n = CS[cc]
                    nc.tensor.transpose(
                        out=ctp[:, CO[cc]:CO[cc] + n],
                        in_=Cns[cc][:n, 128 * fc:128 * (fc + 1)],
                        identity=ident[:n, :n])
                ct = sg.tile([128, C], bf16, tag=f"CnT{fc}", bufs=1)
                nc.vector.tensor_copy(out=ct[:], in_=ctp[:])
                CnTs.append(ct)

            # ---- P3d: sim matmul + simneg -> DRAM ----
            for mc in range(8):
                m = CS[mc]
                sn = cwp.tile([128, C], bf16, tag="snsb")
                for nh in range(2):
                    sp = simps.tile([128, 500], f32, space="PSUM", name=f"sp{mc}_{nh}",
                                    tag="sp")
                    for kc in range(4):
                        nc.tensor.matmul(
                            out=sp[:m, :],
                            lhsT=CnTs[kc][:, CO[mc]:CO[mc] + m],
                            rhs=CnTs[kc][:, 500 * nh:500 * (nh + 1)],
                            start=(kc == 0), stop=(kc == 3))
                    nc.vector.tensor_scalar(
                        out=sn[:m, 500 * nh:500 * (nh + 1)], in0=sp[:m, :],
                        scalar1=-KSIM, scalar2=-KSIM,
                        op0=OP.mult, op1=OP.add)
                nc.sync.dma_start(out=simneg[CO[mc]:CO[mc] + m, :], in_=sn[:m, :])

            simps_cm.__exit__(None, None, None)
            ctps_cm.__exit__(None, None, None)
            # ---- P4: logits passes ----
            for t in range(T):
                xt = xts[t]
                dc = dcp.tile([128, C], bf16)
                nc.scalar.activation(out=dc[:], in_=xt[:], func=AF.Exp,
                                     accum_out=s1col[:, t:t + 1])
                nc.scalar.activation(out=xt[:], in_=xt[:], func=AF.Exp, scale=10.0,
                                     accum_out=s10col[:, t:t + 1])
                rc = cwp.tile([128, 1], f32, tag="rc")
                nc.vector.reciprocal(out=rc[:], in_=s10col[:, t:t + 1])
                g = gpp.tile([128, C], bf16)
                nc.gpsimd.indirect_dma_start(
                    out=g[:], out_offset=None, in_=simneg[:],
                    in_offset=bass.IndirectOffsetOnAxis(ap=labit[:, t:t + 1], axis=0))
                nc.vector.scalar_tensor_tensor(
                    out=xt[:], in0=xt[:], scalar=rc[:, 0:1], in1=g[:],
                    op0=OP.mult, op1=OP.mult)
                dc2 = dcp.tile([128, C], bf16)
                nc.scalar.activation(out=dc2[:], in_=xt[:], func=AF.Ln,
                                     bias=eps1[:, 0:1],
                                     accum_out=wcol[:, t:t + 1])

            # ---- P5: CE gather + final reduction ----
            ceg = sg.tile([128, T], f32)
            logit_flat = bass.AP(logits, 0, [[1, BL * C], [1, 1]])
            for t in range(T):
                nc.gpsimd.indirect_dma_start(
                    out=ceg[:, t:t + 1], out_offset=None, in_=logit_flat,
                    in_offset=bass.IndirectOffsetOnAxis(ap=ceofft[:, t:t + 1], axis=0))
            lnscr = sg.tile([128, T], f32)
            a = sg.tile([128, 4], f32)
            nc.vector.memset(a[:], 0.0)
            nc.scalar.activation(out=lnscr[:], in_=s1col[:], func=AF.Ln,
                                 accum_out=a[:, 0:1])
            nc.vector.tensor_reduce(out=a[:, 1:2], in_=ceg[:],
                                    axis=mybir.AxisListType.X, op=OP.add)
            nc.vector.tensor_reduce(out=a[:, 2:3], in_=wcol[:],
                                    axis=mybir.AxisListType.X, op=OP.add)
            pr = sg.tile([1, 4], f32)
            nc.gpsimd.tensor_reduce(out=pr[:1, :], in_=a[:],
                                    axis=mybir.AxisListType.C, op=OP.add)
            nc.sync.dma_start(out=pin[:], in_=pr[:1, :])
            nc.gpsimd.collective_compute(
                "AllReduce", OP.add, replica_groups=groups,
                ins=[pin.opt()], outs=[pout.opt()])
            pt = sg.tile([1, 4], f32)
            nc.sync.dma_start(out=pt[:1, :], in_=pout[:])
            # loss = (sum_lns1 - sum_xg)/B - 0.1*sum_w/(B*C)
            dl = sg.tile([1, 1], f32)
            nc.vector.tensor_tensor(out=dl[:1, :], in0=pt[:1, 0:1], in1=pt[:1, 1:2],
                                    op=OP.subtract)
            nc.vector.tensor_scalar_mul(dl[:1, :], dl[:1, :], 1.0 / B)
            el = sg.tile([1, 1], f32)
            nc.vector.tensor_scalar_mul(el[:1, :], pt[:1, 2:3], -0.1 / (B * C))
            fl = sg.tile([1, 1], f32)
            nc.vector.tensor_tensor(out=fl[:1, :], in0=dl[:1, :], in1=el[:1, :],
                                    op=OP.add)
            nc.sync.dma_start(out=loss_out[:], in_=fl[:1, :])
    return nc


def _install_patches():
    """Walrus in this container accepts only one sync-wait per instruction:
    split multi-wait instructions into single-wait NOPs."""
    import sys
    import types
    import concourse.tile as tile
    import concourse.mybir as mybir

    if "bass_patches_inline" in sys.modules:
        return

    def split_multi_waits(nc):
        for f in nc.m.functions:
            for bb in f.blocks:
                insts = list(bb.instructions)
                out = []
                changed = False
                for ins in insts:
                    si = getattr(ins, "sync_info", None)
                    waits = list(si.on_wait) if (si is not None and si.on_wait) else []
                    if len(waits) > 1:
                        for w in waits[:-1]:
                            nop = mybir.InstNoOp(
                                name=nc.get_next_instruction_name(),
                                engine=ins.engine)
                            nop.sync_info = mybir.SyncInfo(on_wait=[w], on_update=[])
                            nc.register_instruction(nop)
                            out.append(nop)
                        ins.sync_info = mybir.SyncInfo(
                            on_wait=[waits[-1]], on_update=list(si.on_update or []))
                        changed = True
                    out.append(ins)
                if changed:
                    try:
                        bb.instructions = out
                    except Exception:
                        while len(bb.instructions):
                            bb.instructions.pop()
                        for x in out:
                            bb.instructions.append(x)

    orig_exit = tile.TileContext.__exit__

    def patched_exit(self, exc_type, exc_value, traceback):
        r = orig_exit(self, exc_type, exc_value, traceback)
        if not exc_type:
            split_multi_waits(self.nc)
        return r

    tile.TileContext.__exit__ = patched_exit
    sys.modules["bass_patches_inline"] = types.ModuleType("bass_patches_inline")


def _prep_inputs(logits, features, labels, class_centers):
    logits = np.ascontiguousarray(np.asarray(logits, dtype=np.float32))
    features = np.ascontiguousarray(np.asarray(features, dtype=np.float32))
    labels = np.asarray(labels).astype(np.int64)
    centers = np.ascontiguousarray(np.asarray(class_centers, dtype=np.float32))
    in_maps = []
    for i in range(N_CORES):
        sl = slice(BL * i, BL * (i + 1))
        lab = labels[sl].astype(np.int32)
        labf = lab.reshape(T, 128).T.astype(np.float32).copy()
        labi = lab.reshape(T, 128).T.astype(np.int32).copy()
        ceoff = (np.arange(BL, dtype=np.int64) * C + lab).astype(np.int32)
        ceoff = ceoff.reshape(T, 128).T.copy()
        in_maps.append({
            "logits": np.ascontiguousarray(logits[sl]),
            "features": np.ascontiguousarray(features[sl]),
            "centers": centers,
            "labrow": lab.astype(np.float32).reshape(1, BL),
            "labf": labf,
            "labi": labi,
            "ceoff": ceoff,
            "iotac": np.arange(C, dtype=np.float32).reshape(1, C),
            "iotak": (np.arange(128, dtype=np.float32)[:, None]
                      + 128.0 * np.arange(8, dtype=np.float32)[None, :]),
        })
    return in_maps


def kernel(**inputs):
    _install_patches()
    from concourse.bass_utils import run_bass_kernel_spmd

    if "nc" not in _CACHE:
        _CACHE["nc"] = _build()
    nc = _CACHE["nc"]
    in_maps = _prep_inputs(
        inputs["logits"], inputs["features"], inputs["labels"],
        inputs["class_centers"])
    t0 = time.perf_counter()
    res = run_bass_kernel_spmd(nc, in_maps, list(range(N_CORES)))
    _CACHE["last_wall_ns"] = (time.perf_counter() - t0) * 1e9
    loss = np.asarray(res.results[0]["loss"], dtype=np.float32).reshape(())
    return loss
